# revision 15
# baseline (speedup 1.0000x reference)
"""Trainium2 Bass kernel for nn_Attention_13503377178798.

QLoRA-style attention: NF4-dequant + LoRA linears (q/k/v/o), RoPE, GQA causal
attention. Tensor-parallel over 8 NeuronCores: heads sharded, x replicated,
AllGather of attention output before the o-projection (whose rows are sharded);
the host concatenates the per-core output-column shards.

Device-side NF4 dequant: degree-7 polynomial in (idx - 7.5) evaluated with
fused scalar_tensor_tensor Horner steps on DVE (max table err 2.2e-3), then a
per-64-block absmax scale + bf16 cast on the ACT engine. Weight tiles are
produced in [out, in] layout (scale is per-partition there) and DMA-transposed
(xbar) to the [in, out] layout the PE needs for lhsT.

Attention computes scores transposed (K stationary): scoresT[j,i] tiles give
probsT directly for the PV matmul (no PE transposes); row-sums come from a
ones-vector matmul accumulated on PSUM, and the softmax normalization is
applied to the (small) attention output via a PE-broadcast reciprocal.
"""

import sys

import numpy as np

for _p in ("/opt/trn_rl_repo",):
    if _p not in sys.path:
        sys.path.insert(0, _p)

import concourse.bass as bass  # noqa: E402
import concourse.mybir as mybir  # noqa: E402
import concourse.tile as tile  # noqa: E402
from concourse import bacc  # noqa: E402

S = 2048
D = 4096
N_HEADS, N_KV_HEADS, HEAD_DIM = 32, 8, 128
BLOCK = 64
LORA_SCALING = 2.0
NCORES = 8
HPC = N_HEADS // NCORES  # 4 q heads per core
QO = HPC * HEAD_DIM      # 512 q rows per core
KVO = 128                # 1 kv head per core
SM_SCALE = 1.0 / float(np.sqrt(HEAD_DIM))

NF4 = np.array([-1.0, -0.6961928009986877, -0.5250730514526367, -0.39491748809814453,
                -0.28444138169288635, -0.18477343022823334, -0.09105003625154495, 0.0,
                0.07958029955625534, 0.16093020141124725, 0.24611230194568634,
                0.33791524171829224, 0.44070982933044434, 0.5626170039176941,
                0.7229568362236023, 1.0])
_t16 = np.arange(16) - 7.5
POLY = np.polyfit(_t16, NF4, 7)[::-1]  # POLY[k] = coeff of t^k

AF = mybir.ActivationFunctionType
ALU = mybir.AluOpType
DT = mybir.dt

N_QKV = 6   # dequant o-tiles: 0..3 q heads, 4 k, 5 v
N_WO = 4    # o-tiles 6..9: wo row shard


def build_graph(s=S):
    """One SPMD graph; per-core differences live in the input shards."""
    n_sc = s // 512
    n_it = s // 128
    KT = D // 128
    KH = KT // 2

    nc = bacc.Bacc(None, target_bir_lowering=False, debug=False)

    xT = nc.declare_dram_parameter("xT", [D, s], DT.float32, isOutput=False)
    qkvo_idx = nc.declare_dram_parameter("qkvo_idx", [2 * QO + 2 * KVO, D], DT.int32, isOutput=False)
    qkvo_am = nc.declare_dram_parameter("qkvo_am", [2 * QO + 2 * KVO, BLOCK], DT.float32, isOutput=False)
    acombT = nc.declare_dram_parameter("acombT", [D, 80], DT.float32, isOutput=False)
    aoT = nc.declare_dram_parameter("aoT", [D, 16], DT.float32, isOutput=False)
    bqT = nc.declare_dram_parameter("bqT", [16, QO], DT.float32, isOutput=False)
    bkT = nc.declare_dram_parameter("bkT", [16, KVO], DT.float32, isOutput=False)
    bvT = nc.declare_dram_parameter("bvT", [16, KVO], DT.float32, isOutput=False)
    boT = nc.declare_dram_parameter("boT", [16, QO], DT.float32, isOutput=False)
    cosT = nc.declare_dram_parameter("cosT", [128, s], DT.float32, isOutput=False)
    sinT = nc.declare_dram_parameter("sinT", [128, s], DT.float32, isOutput=False)
    maskTd = nc.declare_dram_parameter("maskTd", [128, 128], DT.float32, isOutput=False)
    out = nc.declare_dram_parameter("out", [QO, s], DT.float32, isOutput=True)

    with tile.TileContext(nc) as tc:
        import contextlib
        ctx = contextlib.ExitStack()
        with ctx:
            const = ctx.enter_context(tc.tile_pool(name="const", bufs=1))
            wts = ctx.enter_context(tc.tile_pool(name="wts", bufs=1))
            deq_io = ctx.enter_context(tc.tile_pool(name="deq_io", bufs=2))
            deq_tmp = ctx.enter_context(tc.tile_pool(name="deq_tmp", bufs=1))
            xin = ctx.enter_context(tc.tile_pool(name="xin", bufs=2))
            work = ctx.enter_context(tc.tile_pool(name="work", bufs=2))
            dram = ctx.enter_context(tc.tile_pool(name="dram", bufs=1, space="DRAM"))
            psum = ctx.enter_context(tc.tile_pool(name="psum", bufs=2, space="PSUM"))
            psum_acc = ctx.enter_context(tc.tile_pool(name="psum_acc", bufs=2, space="PSUM"))
            psum_small = ctx.enter_context(tc.tile_pool(name="psum_small", bufs=1, space="PSUM"))

            # ---- constants ----
            cos_sb = const.tile([128, s], DT.bfloat16, tag="cos")
            sin_sb = const.tile([128, s], DT.bfloat16, tag="sin")
            nc.gpsimd.dma_start(out=cos_sb[:, :], in_=cosT[:, :])
            nc.gpsimd.dma_start(out=sin_sb[:, :], in_=sinT[:, :])
            maskT_sb = const.tile([128, 128], DT.float32, tag="mask")
            nc.sync.dma_start(out=maskT_sb[:, :], in_=maskTd[:, :])
            ones_col = const.tile([128, 1], DT.bfloat16, tag="ones_col")
            nc.vector.memset(ones_col[:, :], 1.0)
            ones_row = const.tile([1, 128], DT.bfloat16, tag="ones_row")
            nc.vector.memset(ones_row[:, :], 1.0)

            acomb_sb = const.tile([128, KT, 80], DT.bfloat16, tag="acomb")
            nc.gpsimd.dma_start(out=acomb_sb[:, :, :],
                                in_=acombT.ap().rearrange("(kt p) r -> p kt r", p=128)[:, :, :])
            ao_sb = const.tile([128, KT, 16], DT.bfloat16, tag="ao")
            nc.gpsimd.dma_start(out=ao_sb[:, :, :],
                                in_=aoT.ap().rearrange("(kt p) r -> p kt r", p=128)[:, :, :])
            bq_sb = const.tile([16, QO], DT.bfloat16, tag="bq")
            nc.gpsimd.dma_start(out=bq_sb[:, :], in_=bqT[:, :])
            bk_sb = const.tile([16, KVO], DT.bfloat16, tag="bk")
            nc.gpsimd.dma_start(out=bk_sb[:, :], in_=bkT[:, :])
            bv_sb = const.tile([16, KVO], DT.bfloat16, tag="bv")
            nc.gpsimd.dma_start(out=bv_sb[:, :], in_=bvT[:, :])
            bo_sb = const.tile([16, QO], DT.bfloat16, tag="bo")
            nc.gpsimd.dma_start(out=bo_sb[:, :], in_=boT[:, :])

            # ---- dequant one o-tile (128 rows of W) -> W^T k-tiles ----
            def dequant_otile(ot, wt_tile, eng=None):
                am_sb = deq_tmp.tile([128, BLOCK], DT.float32, tag="am")
                nc.sync.dma_start(out=am_sb[:, :], in_=qkvo_am[128 * ot:128 * (ot + 1), :])
                for q in range(4):
                    e = nc.vector
                    be = nc.gpsimd
                    idx_sb = deq_io.tile([128, 1024], DT.int32, tag="idx")
                    nc.sync.dma_start(out=idx_sb[:, :],
                                      in_=qkvo_idx[128 * ot:128 * (ot + 1), 1024 * q:1024 * (q + 1)])
                    # per-64-block absmax scales, broadcast on the other engine
                    scl_sb = deq_tmp.tile([128, 1024], DT.bfloat16, tag="scl")
                    be.tensor_copy(
                        scl_sb[:, :].rearrange("p (b i) -> p b i", b=16),
                        am_sb[:, 16 * q:16 * (q + 1)].broadcast_to((128, 16, 64)))
                    t_sb = deq_tmp.tile([128, 1024], DT.float32, tag="deq_t")
                    nc.scalar.activation(out=t_sb[:, :], in_=idx_sb[:, :],
                                         func=AF.Copy, bias=-7.5)
                    acc_sb = deq_tmp.tile([128, 1024], DT.float32, tag="deq_acc")
                    e.tensor_scalar(out=acc_sb[:, :], in0=t_sb[:, :],
                                    scalar1=float(POLY[7]), scalar2=None, op0=ALU.mult)
                    for k in range(6, 0, -1):
                        e.scalar_tensor_tensor(out=acc_sb[:, :], in0=acc_sb[:, :],
                                               scalar=float(POLY[k]), in1=t_sb[:, :],
                                               op0=ALU.add, op1=ALU.mult)
                    w_bf = deq_tmp.tile([128, 1024], DT.bfloat16, tag="w_bf")
                    e.scalar_tensor_tensor(out=w_bf[:, :], in0=acc_sb[:, :],
                                           scalar=float(POLY[0]), in1=scl_sb[:, :],
                                           op0=ALU.add, op1=ALU.mult)
                    nc.sync.dma_start_transpose(out=wt_tile[:, 8 * q:8 * (q + 1), :],
                                                in_=w_bf[:, :])

            wt_qkv = [wts.tile([128, KT, 128], DT.bfloat16, tag=f"wt{i}", name=f"wt{i}")
                      for i in range(N_QKV)]
            for ot in range(N_QKV):
                dequant_otile(ot, wt_qkv[ot])

            # ---- projections (per s-chunk of 512) ----
            qT = [const.tile([128, s], DT.bfloat16, tag=f"qT{h}", name=f"qT{h}") for h in range(HPC)]
            kT = const.tile([128, s], DT.bfloat16, tag="kT")
            v_sb = const.tile([128, n_it, 128], DT.bfloat16, tag="v")

            def rope(dst, raw, sc):
                sl = slice(512 * sc, 512 * (sc + 1))
                cs_lo, cs_hi = cos_sb[0:64, sl], cos_sb[64:128, sl]
                sn_lo, sn_hi = sin_sb[0:64, sl], sin_sb[64:128, sl]
                re, ro = raw[0:64, :], raw[64:128, :]
                t1 = work.tile([64, 512], DT.bfloat16, tag="rope1", name="t1")
                t2 = work.tile([64, 512], DT.bfloat16, tag="rope2", name="t2")
                nc.vector.tensor_tensor(out=t1[:, :], in0=re, in1=cs_lo, op=ALU.mult)
                nc.vector.tensor_tensor(out=t2[:, :], in0=ro, in1=sn_hi, op=ALU.mult)
                nc.vector.tensor_tensor(out=dst[0:64, sl],
                                        in0=t1[:, :], in1=t2[:, :], op=ALU.subtract)
                t3 = work.tile([64, 512], DT.bfloat16, tag="rope3", name="t3")
                t4 = work.tile([64, 512], DT.bfloat16, tag="rope4", name="t4")
                nc.vector.tensor_tensor(out=t3[:, :], in0=re, in1=sn_lo, op=ALU.mult)
                nc.vector.tensor_tensor(out=t4[:, :], in0=ro, in1=cs_hi, op=ALU.mult)
                nc.vector.tensor_tensor(out=dst[64:128, sl],
                                        in0=t3[:, :], in1=t4[:, :], op=ALU.add)

            for sc in range(n_sc):
                xbf = [xin.tile([128, KH, 512], DT.bfloat16, tag="xbf", name=f"xbf{sc}_{h}")
                       for h in range(2)]
                for half in range(2):
                    nc.gpsimd.dma_start(
                        out=xbf[half][:, :, :],
                        in_=xT.ap().rearrange("(kt p) s -> p kt s", p=128)[
                            :, KH * half:KH * (half + 1), 512 * sc:512 * (sc + 1)])

                p_l1 = psum_small.tile([80, 512], DT.float32, tag="p_small", name="p_l1")
                for half in range(2):
                    for j in range(KH):
                        kt = KH * half + j
                        nc.tensor.matmul(p_l1[:, :], lhsT=acomb_sb[:, kt, :], rhs=xbf[half][:, j, :],
                                         start=(kt == 0), stop=(kt == KT - 1), skip_group_check=True)
                l1q_sb = work.tile([16, 512], DT.bfloat16, tag="l1q")
                l1k_sb = work.tile([16, 512], DT.bfloat16, tag="l1k")
                l1v_sb = work.tile([16, 512], DT.bfloat16, tag="l1v")
                nc.scalar.activation(out=l1q_sb[:, :], in_=p_l1[0:16, :], func=AF.Copy, scale=LORA_SCALING)
                nc.scalar.activation(out=l1k_sb[:, :], in_=p_l1[32:48, :], func=AF.Copy, scale=LORA_SCALING)
                nc.scalar.activation(out=l1v_sb[:, :], in_=p_l1[64:80, :], func=AF.Copy, scale=LORA_SCALING)

                for ot in range(6):  # q0..q3, k, v -> psum [o=128, s=512]
                    ps = psum.tile([128, 512], DT.float32, tag="p_proj", name=f"ps{sc}_{ot}")
                    for half in range(2):
                        for j in range(KH):
                            kt = KH * half + j
                            nc.tensor.matmul(ps[:, :], lhsT=wt_qkv[ot][:, kt, :], rhs=xbf[half][:, j, :],
                                             start=(kt == 0), stop=False, skip_group_check=True)
                    if ot < 4:
                        nc.tensor.matmul(ps[:, :], lhsT=bq_sb[:, 128 * ot:128 * (ot + 1)],
                                         rhs=l1q_sb[:, :], start=False, stop=True, skip_group_check=True)
                    elif ot == 4:
                        nc.tensor.matmul(ps[:, :], lhsT=bk_sb[:, :], rhs=l1k_sb[:, :],
                                         start=False, stop=True, skip_group_check=True)
                    else:
                        nc.tensor.matmul(ps[:, :], lhsT=bv_sb[:, :], rhs=l1v_sb[:, :],
                                         start=False, stop=True, skip_group_check=True)
                    raw = work.tile([128, 512], DT.bfloat16, tag="praw", name=f"raw{sc}_{ot}")
                    nc.scalar.activation(out=raw[:, :], in_=ps[:, :], func=AF.Copy)
                    if ot < 5:
                        rope(qT[ot] if ot < 4 else kT, raw, sc)
                    else:
                        nc.sync.dma_start_transpose(out=v_sb[:, 4 * sc:4 * (sc + 1), :],
                                                    in_=raw[:, :])

            # ---- wo weight tiles (dequant interleaved with attention below) ----
            wt_wo = [wts.tile([128, KT, 128], DT.bfloat16, tag=f"wt{i}", name=f"wo{i}")
                     for i in range(N_WO)]

            # ---- attention (scores transposed; probsT feeds PV directly) ----
            # chunked over i: after each 512-column chunk finishes on all
            # heads, its AllGather is issued and the o-projection for that
            # chunk follows — comm and o-proj overlap later attention chunks.
            ag_in_c = [dram.tile([QO, 512], DT.bfloat16, tag=f"ag_in{ic}", name=f"ag_in{ic}")
                       for ic in range(n_sc)]
            ag_out_c = [dram.tile([NCORES * QO, 512], DT.bfloat16, tag=f"ag_out{ic}",
                                  name=f"ag_out{ic}", addr_space="Shared")
                        for ic in range(n_sc)]

            def attn_chunk(h, ic):
                i0 = 512 * ic
                a_ps = psum_acc.tile([128, 512], DT.float32, tag="p_attn", name=f"a{h}_{ic}")
                r_ps = psum_acc.tile([1, 512], DT.float32, tag="p_rows", name=f"r{h}_{ic}")
                jmax = 4 * ic + 3
                for jt in range(jmax + 1):
                    off = max(0, 128 * (jt - 4 * ic))
                    s_ps = psum.tile([128, 512], DT.float32, tag="p_proj", name=f"s{h}_{ic}_{jt}")
                    nc.tensor.matmul(s_ps[:, off:512], lhsT=kT[:, 128 * jt:128 * (jt + 1)],
                                     rhs=qT[h][:, i0 + off:i0 + 512], start=True, stop=True,
                                     skip_group_check=True)
                    if jt >= 4 * ic:  # diagonal block: additive causal mask (transposed)
                        nc.vector.tensor_tensor(out=s_ps[:, off:off + 128],
                                                in0=s_ps[:, off:off + 128],
                                                in1=maskT_sb[:, :], op=ALU.add)
                    pT = work.tile([128, 512], DT.bfloat16, tag="probsT", name=f"pT{h}_{ic}_{jt}")
                    nc.scalar.activation(out=pT[:, off:512], in_=s_ps[:, off:512],
                                         func=AF.Exp, scale=SM_SCALE)
                    nc.tensor.matmul(r_ps[:, off:512], lhsT=ones_col[:, :], rhs=pT[:, off:512],
                                     start=(jt == 0), stop=(jt == jmax), skip_group_check=True)
                    nc.tensor.matmul(a_ps[:, off:512], lhsT=v_sb[:, jt, :], rhs=pT[:, off:512],
                                     start=(jt == 0), stop=(jt == jmax), skip_group_check=True)
                rs_sb = work.tile([1, 512], DT.float32, tag="rs")
                nc.scalar.activation(out=rs_sb[:, :], in_=r_ps[:, :], func=AF.Copy)
                rc_sb = work.tile([1, 512], DT.float32, tag="rc")
                nc.vector.reciprocal(rc_sb[:, :], rs_sb[:, :])
                rcb = work.tile([1, 512], DT.bfloat16, tag="rcb")
                nc.vector.tensor_copy(rcb[:, :], rc_sb[:, :])
                b_ps = psum_small.tile([128, 512], DT.float32, tag="p_small", name=f"b{h}_{ic}")
                nc.tensor.matmul(b_ps[:, :], lhsT=ones_row[:, :], rhs=rcb[:, :],
                                 start=True, stop=True, skip_group_check=True)
                bc_sb = work.tile([128, 512], DT.float32, tag="bc")
                nc.vector.tensor_copy(bc_sb[:, :], b_ps[:, :])
                at_sb = work.tile([128, 512], DT.bfloat16, tag="attn")
                nc.vector.tensor_tensor(out=at_sb[:, :], in0=a_ps[:, :], in1=bc_sb[:, :],
                                        op=ALU.mult)
                nc.sync.dma_start(out=ag_in_c[ic][128 * h:128 * (h + 1), :], in_=at_sb[:, :])

            def oproj_chunk(sc):
                ag_r = ag_out_c[sc].rearrange("(kt p) s -> p kt s", p=128)
                gbf = [xin.tile([128, KH, 512], DT.bfloat16, tag="xbf", name=f"gbf{sc}_{h}")
                       for h in range(2)]
                for half in range(2):
                    nc.sync.dma_start(out=gbf[half][:, :, :],
                                      in_=ag_r[:, KH * half:KH * (half + 1), :])
                p_lo = psum_small.tile([16, 512], DT.float32, tag="p_small", name=f"p_lo{sc}")
                for half in range(2):
                    for j in range(KH):
                        kt = KH * half + j
                        nc.tensor.matmul(p_lo[:, :], lhsT=ao_sb[:, kt, :], rhs=gbf[half][:, j, :],
                                         start=(kt == 0), stop=(kt == KT - 1), skip_group_check=True)
                lo_sb = work.tile([16, 512], DT.bfloat16, tag="l1", name=f"lo{sc}")
                nc.scalar.activation(out=lo_sb[:, :], in_=p_lo[:, :], func=AF.Copy, scale=LORA_SCALING)
                for ot in range(N_WO):
                    ps = psum.tile([128, 512], DT.float32, tag="p_proj", name=f"py{sc}_{ot}")
                    for half in range(2):
                        for j in range(KH):
                            kt = KH * half + j
                            nc.tensor.matmul(ps[:, :], lhsT=wt_wo[ot][:, kt, :], rhs=gbf[half][:, j, :],
                                             start=(kt == 0), stop=False, skip_group_check=True)
                    nc.tensor.matmul(ps[:, :], lhsT=bo_sb[:, 128 * ot:128 * (ot + 1)], rhs=lo_sb[:, :],
                                     start=False, stop=True, skip_group_check=True)
                    y_sb = work.tile([128, 512], DT.float32, tag="y", name=f"y{sc}_{ot}")
                    nc.scalar.activation(out=y_sb[:, :], in_=ps[:, :], func=AF.Copy)
                    nc.sync.dma_start(out=out[128 * ot:128 * (ot + 1), 512 * sc:512 * (sc + 1)],
                                      in_=y_sb[:, :])

            # wo dequant is front-loaded into the first attention chunks; the
            # o-projection of early chunks starts before the last attention
            # chunk so the final AllGather hides behind compute.
            wo_sched = [[] for _ in range(n_sc)]
            for i in range(N_WO):
                wo_sched[min(i // 2, n_sc - 1)].append(i)
            op_sched = [[] for _ in range(n_sc)]
            for sc in range(n_sc):
                op_sched[min(sc + 2, n_sc - 1)].append(sc)
            for ic in range(n_sc):
                for h in range(HPC):
                    attn_chunk(h, ic)
                nc.gpsimd.collective_compute(
                    "AllGather", ALU.bypass,
                    replica_groups=[list(range(NCORES))],
                    ins=[ag_in_c[ic][:, :].opt()],
                    outs=[ag_out_c[ic][:, :].opt()],
                )
                for i in wo_sched[ic]:
                    dequant_otile(N_QKV + i, wt_wo[i])
                for sc in op_sched[ic]:
                    oproj_chunk(sc)
    nc.compile()
    return nc


# ---------------- host side ----------------

_PERM128 = np.concatenate([np.arange(0, 128, 2), np.arange(1, 128, 2)])


def _perm_rows(n_heads):
    return np.concatenate([128 * h + _PERM128 for h in range(n_heads)])


def shard_inputs(x, cos, sin, mask, wq_idx, wq_absmax, wq_A, wq_B,
                 wk_idx, wk_absmax, wk_A, wk_B, wv_idx, wv_absmax, wv_A, wv_B,
                 wo_idx, wo_absmax, wo_A, wo_B):
    s = np.asarray(x).shape[1]
    xT = np.ascontiguousarray(np.asarray(x, dtype=np.float32).reshape(s, D).T)
    cosT = np.asarray(cos, dtype=np.float32).T
    sinT = np.asarray(sin, dtype=np.float32).T
    # RoPE pairs de-interleaved to [evens(64); odds(64)]: both halves use the
    # same per-pair angle table.
    cosE = np.ascontiguousarray(np.concatenate([cosT, cosT], axis=0))
    sinE = np.ascontiguousarray(np.concatenate([sinT, sinT], axis=0))
    maskTd = np.ascontiguousarray(np.asarray(mask, dtype=np.float32)[0:128, 0:128].T)

    am_q = np.asarray(wq_absmax, dtype=np.float32).reshape(D, BLOCK)
    am_k = np.asarray(wk_absmax, dtype=np.float32).reshape(N_KV_HEADS * HEAD_DIM, BLOCK)
    am_v = np.asarray(wv_absmax, dtype=np.float32).reshape(N_KV_HEADS * HEAD_DIM, BLOCK)
    am_o = np.asarray(wo_absmax, dtype=np.float32).reshape(D, BLOCK)
    pq = _perm_rows(HPC)
    pk = _perm_rows(1)
    acomb = np.zeros((80, D), dtype=np.float32)
    acomb[0:16] = wq_A
    acomb[32:48] = wk_A
    acomb[64:80] = wv_A
    acombT = np.ascontiguousarray(acomb.T)
    aoT = np.ascontiguousarray(np.asarray(wo_A, dtype=np.float32).T)

    in_maps = []
    for m in range(NCORES):
        qs, ks = slice(QO * m, QO * (m + 1)), slice(KVO * m, KVO * (m + 1))
        idx = np.concatenate([
            np.asarray(wq_idx)[qs][pq], np.asarray(wk_idx)[ks][pk],
            np.asarray(wv_idx)[ks], np.asarray(wo_idx)[qs]], axis=0)
        am = np.concatenate([am_q[qs][pq], am_k[ks][pk], am_v[ks], am_o[qs]], axis=0)
        in_maps.append({
            "xT": xT,
            "qkvo_idx": np.ascontiguousarray(idx.astype(np.int32)),
            "qkvo_am": np.ascontiguousarray(am),
            "acombT": acombT,
            "aoT": aoT,
            "bqT": np.ascontiguousarray(np.asarray(wq_B, dtype=np.float32)[qs][pq].T),
            "bkT": np.ascontiguousarray(np.asarray(wk_B, dtype=np.float32)[ks][pk].T),
            "bvT": np.ascontiguousarray(np.asarray(wv_B, dtype=np.float32)[ks].T),
            "boT": np.ascontiguousarray(np.asarray(wo_B, dtype=np.float32)[qs].T),
            "cosT": cosE,
            "sinT": sinE,
            "maskTd": maskTd,
        })
    return in_maps


def assemble(results, s):
    y = np.concatenate([np.asarray(results[m]["out"]).T for m in range(NCORES)], axis=1)
    return np.ascontiguousarray(y.reshape(1, s, D).astype(np.float32))


_CACHED = {}


def _install_ntff_hook():
    """The agent image's antenv lacks axon_hooks; synthesize it so
    run_bass_kernel_spmd(trace=True) can capture NTFF profiles."""
    import sys as _sys
    import types as _types
    if "antenv.axon_hooks" in _sys.modules:
        return
    try:
        from trn_agent_boot.trn_boot import _ntff_profile_via_ctypes
        hook = _ntff_profile_via_ctypes("/opt/axon/libaxon_pjrt.so")
    except Exception:
        hook = None
    mod = _types.ModuleType("antenv.axon_hooks")
    mod._hook = hook
    mod.get_axon_ntff_profile_hook = lambda: mod._hook
    mod.set_axon_ntff_profile_hook = lambda h: setattr(mod, "_hook", h)
    _sys.modules["antenv.axon_hooks"] = mod
    try:
        import antenv
        antenv.axon_hooks = mod
    except Exception:
        pass


def kernel(**inputs):
    from concourse.bass_utils import run_bass_kernel_spmd

    trace = bool(_CACHED.pop("trace", False))
    if trace:
        _install_ntff_hook()
    in_maps = shard_inputs(**inputs)
    s = np.asarray(inputs["x"]).shape[1]
    key = ("nc", s)
    if key not in _CACHED:
        _CACHED[key] = build_graph(s)
    nc = _CACHED[key]
    res = run_bass_kernel_spmd(nc, in_maps, core_ids=list(range(NCORES)), trace=trace)
    if trace:
        _CACHED["last_res"] = res
    return assemble(res.results, s)


# revision 16
# speedup vs baseline: 1.0525x; 1.0525x over previous
"""Trainium2 Bass kernel for nn_Attention_13503377178798.

QLoRA-style attention: NF4-dequant + LoRA linears (q/k/v/o), RoPE, GQA causal
attention. Tensor-parallel over 8 NeuronCores: heads sharded, x replicated,
AllGather of attention output before the o-projection (whose rows are sharded);
the host concatenates the per-core output-column shards.

Device-side NF4 dequant: degree-7 polynomial in (idx - 7.5) evaluated with
fused scalar_tensor_tensor Horner steps on DVE (max table err 2.2e-3), then a
per-64-block absmax scale + bf16 cast on the ACT engine. Weight tiles are
produced in [out, in] layout (scale is per-partition there) and DMA-transposed
(xbar) to the [in, out] layout the PE needs for lhsT.

Attention computes scores transposed (K stationary): scoresT[j,i] tiles give
probsT directly for the PV matmul (no PE transposes); row-sums come from a
ones-vector matmul accumulated on PSUM, and the softmax normalization is
applied to the (small) attention output via a PE-broadcast reciprocal.
"""

import sys

import numpy as np

for _p in ("/opt/trn_rl_repo",):
    if _p not in sys.path:
        sys.path.insert(0, _p)

import concourse.bass as bass  # noqa: E402
import concourse.mybir as mybir  # noqa: E402
import concourse.tile as tile  # noqa: E402
from concourse import bacc  # noqa: E402

S = 2048
D = 4096
N_HEADS, N_KV_HEADS, HEAD_DIM = 32, 8, 128
BLOCK = 64
LORA_SCALING = 2.0
NCORES = 8
HPC = N_HEADS // NCORES  # 4 q heads per core
QO = HPC * HEAD_DIM      # 512 q rows per core
KVO = 128                # 1 kv head per core
SM_SCALE = 1.0 / float(np.sqrt(HEAD_DIM))

NF4 = np.array([-1.0, -0.6961928009986877, -0.5250730514526367, -0.39491748809814453,
                -0.28444138169288635, -0.18477343022823334, -0.09105003625154495, 0.0,
                0.07958029955625534, 0.16093020141124725, 0.24611230194568634,
                0.33791524171829224, 0.44070982933044434, 0.5626170039176941,
                0.7229568362236023, 1.0])
_t16 = np.arange(16) - 7.5
POLY = np.polyfit(_t16, NF4, 7)[::-1]  # POLY[k] = coeff of t^k

AF = mybir.ActivationFunctionType
ALU = mybir.AluOpType
DT = mybir.dt

N_QKV = 6   # dequant o-tiles: 0..3 q heads, 4 k, 5 v
N_WO = 4    # o-tiles 6..9: wo row shard


def build_graph(s=S):
    """One SPMD graph; per-core differences live in the input shards."""
    n_sc = s // 512
    n_it = s // 128
    KT = D // 128
    KH = KT // 2

    nc = bacc.Bacc(None, target_bir_lowering=False, debug=False)

    xT = nc.declare_dram_parameter("xT", [D, s], DT.float32, isOutput=False)
    qkvo_idx = nc.declare_dram_parameter("qkvo_idx", [2 * QO + 2 * KVO, D], DT.int32, isOutput=False)
    qkvo_am = nc.declare_dram_parameter("qkvo_am", [2 * QO + 2 * KVO, BLOCK], DT.float32, isOutput=False)
    acombT = nc.declare_dram_parameter("acombT", [D, 80], DT.float32, isOutput=False)
    aoT = nc.declare_dram_parameter("aoT", [D, 16], DT.float32, isOutput=False)
    bqT = nc.declare_dram_parameter("bqT", [16, QO], DT.float32, isOutput=False)
    bkT = nc.declare_dram_parameter("bkT", [16, KVO], DT.float32, isOutput=False)
    bvT = nc.declare_dram_parameter("bvT", [16, KVO], DT.float32, isOutput=False)
    boT = nc.declare_dram_parameter("boT", [16, QO], DT.float32, isOutput=False)
    cosT = nc.declare_dram_parameter("cosT", [128, s], DT.float32, isOutput=False)
    sinT = nc.declare_dram_parameter("sinT", [128, s], DT.float32, isOutput=False)
    maskTd = nc.declare_dram_parameter("maskTd", [128, 128], DT.float32, isOutput=False)
    out = nc.declare_dram_parameter("out", [QO, s], DT.float32, isOutput=True)

    with tile.TileContext(nc) as tc:
        import contextlib
        ctx = contextlib.ExitStack()
        with ctx:
            const = ctx.enter_context(tc.tile_pool(name="const", bufs=1))
            wts = ctx.enter_context(tc.tile_pool(name="wts", bufs=1))
            deq_io = ctx.enter_context(tc.tile_pool(name="deq_io", bufs=2))
            deq_tmp = ctx.enter_context(tc.tile_pool(name="deq_tmp", bufs=2))
            xin = ctx.enter_context(tc.tile_pool(name="xin", bufs=2))
            work = ctx.enter_context(tc.tile_pool(name="work", bufs=2))
            dram = ctx.enter_context(tc.tile_pool(name="dram", bufs=1, space="DRAM"))
            psum = ctx.enter_context(tc.tile_pool(name="psum", bufs=2, space="PSUM"))
            psum_acc = ctx.enter_context(tc.tile_pool(name="psum_acc", bufs=2, space="PSUM"))
            psum_small = ctx.enter_context(tc.tile_pool(name="psum_small", bufs=1, space="PSUM"))

            # ---- constants ----
            cos_sb = const.tile([128, s], DT.bfloat16, tag="cos")
            sin_sb = const.tile([128, s], DT.bfloat16, tag="sin")
            nc.gpsimd.dma_start(out=cos_sb[:, :], in_=cosT[:, :])
            nc.gpsimd.dma_start(out=sin_sb[:, :], in_=sinT[:, :])
            maskT_sb = const.tile([128, 128], DT.float32, tag="mask")
            nc.sync.dma_start(out=maskT_sb[:, :], in_=maskTd[:, :])
            ones_col = const.tile([128, 1], DT.bfloat16, tag="ones_col")
            nc.vector.memset(ones_col[:, :], 1.0)
            ones_row = const.tile([1, 128], DT.bfloat16, tag="ones_row")
            nc.vector.memset(ones_row[:, :], 1.0)

            acomb_sb = const.tile([128, KT, 80], DT.bfloat16, tag="acomb")
            nc.gpsimd.dma_start(out=acomb_sb[:, :, :],
                                in_=acombT.ap().rearrange("(kt p) r -> p kt r", p=128)[:, :, :])
            ao_sb = const.tile([128, KT, 16], DT.bfloat16, tag="ao")
            nc.gpsimd.dma_start(out=ao_sb[:, :, :],
                                in_=aoT.ap().rearrange("(kt p) r -> p kt r", p=128)[:, :, :])
            bq_sb = const.tile([16, QO], DT.bfloat16, tag="bq")
            nc.gpsimd.dma_start(out=bq_sb[:, :], in_=bqT[:, :])
            bk_sb = const.tile([16, KVO], DT.bfloat16, tag="bk")
            nc.gpsimd.dma_start(out=bk_sb[:, :], in_=bkT[:, :])
            bv_sb = const.tile([16, KVO], DT.bfloat16, tag="bv")
            nc.gpsimd.dma_start(out=bv_sb[:, :], in_=bvT[:, :])
            bo_sb = const.tile([16, QO], DT.bfloat16, tag="bo")
            nc.gpsimd.dma_start(out=bo_sb[:, :], in_=boT[:, :])

            # ---- dequant one o-tile (128 rows of W) -> W^T k-tiles ----
            def dequant_otile(ot, wt_tile, eng=None):
                am_sb = deq_tmp.tile([128, BLOCK], DT.float32, tag="am")
                nc.sync.dma_start(out=am_sb[:, :], in_=qkvo_am[128 * ot:128 * (ot + 1), :])
                for q in range(4):
                    e = nc.vector
                    idx_sb = deq_io.tile([128, 1024], DT.int32, tag="idx")
                    nc.sync.dma_start(out=idx_sb[:, :],
                                      in_=qkvo_idx[128 * ot:128 * (ot + 1), 1024 * q:1024 * (q + 1)])
                    t_sb = deq_tmp.tile([128, 1024], DT.float32, tag="deq_t")
                    nc.scalar.activation(out=t_sb[:, :], in_=idx_sb[:, :],
                                         func=AF.Copy, bias=-7.5)
                    acc_sb = deq_tmp.tile([128, 1024], DT.float32, tag="deq_acc")
                    e.tensor_scalar(out=acc_sb[:, :], in0=t_sb[:, :],
                                    scalar1=float(POLY[7]), scalar2=None, op0=ALU.mult)
                    for k in range(6, 0, -1):
                        e.scalar_tensor_tensor(out=acc_sb[:, :], in0=acc_sb[:, :],
                                               scalar=float(POLY[k]), in1=t_sb[:, :],
                                               op0=ALU.add, op1=ALU.mult)
                    w_bf = deq_tmp.tile([128, 1024], DT.bfloat16, tag="w_bf")
                    # absmax applied via a stride-0 broadcast read (per 64-block)
                    e.scalar_tensor_tensor(out=w_bf[:, :].rearrange("p (b i) -> p b i", b=16),
                                           in0=acc_sb[:, :].rearrange("p (b i) -> p b i", b=16),
                                           scalar=float(POLY[0]),
                                           in1=am_sb[:, 16 * q:16 * (q + 1)].broadcast_to((128, 16, 64)),
                                           op0=ALU.add, op1=ALU.mult)
                    nc.sync.dma_start_transpose(out=wt_tile[:, 8 * q:8 * (q + 1), :],
                                                in_=w_bf[:, :])

            wt_qkv = [wts.tile([128, KT, 128], DT.bfloat16, tag=f"wt{i}", name=f"wt{i}")
                      for i in range(N_QKV)]
            for ot in range(N_QKV):
                dequant_otile(ot, wt_qkv[ot])

            # ---- projections (per s-chunk of 512) ----
            qT = [const.tile([128, s], DT.bfloat16, tag=f"qT{h}", name=f"qT{h}") for h in range(HPC)]
            kT = const.tile([128, s], DT.bfloat16, tag="kT")
            v_sb = const.tile([128, n_it, 128], DT.bfloat16, tag="v")

            def rope(dst, raw, sc):
                sl = slice(512 * sc, 512 * (sc + 1))
                cs_lo, cs_hi = cos_sb[0:64, sl], cos_sb[64:128, sl]
                sn_lo, sn_hi = sin_sb[0:64, sl], sin_sb[64:128, sl]
                re, ro = raw[0:64, :], raw[64:128, :]
                t1 = work.tile([64, 512], DT.bfloat16, tag="rope1", name="t1")
                t2 = work.tile([64, 512], DT.bfloat16, tag="rope2", name="t2")
                nc.gpsimd.tensor_tensor(out=t1[:, :], in0=re, in1=cs_lo, op=ALU.mult)
                nc.gpsimd.tensor_tensor(out=t2[:, :], in0=ro, in1=sn_hi, op=ALU.mult)
                nc.gpsimd.tensor_tensor(out=dst[0:64, sl],
                                        in0=t1[:, :], in1=t2[:, :], op=ALU.subtract)
                t3 = work.tile([64, 512], DT.bfloat16, tag="rope3", name="t3")
                t4 = work.tile([64, 512], DT.bfloat16, tag="rope4", name="t4")
                nc.gpsimd.tensor_tensor(out=t3[:, :], in0=re, in1=sn_lo, op=ALU.mult)
                nc.gpsimd.tensor_tensor(out=t4[:, :], in0=ro, in1=cs_hi, op=ALU.mult)
                nc.gpsimd.tensor_tensor(out=dst[64:128, sl],
                                        in0=t3[:, :], in1=t4[:, :], op=ALU.add)

            for sc in range(n_sc):
                xbf = [xin.tile([128, KH, 512], DT.bfloat16, tag="xbf", name=f"xbf{sc}_{h}")
                       for h in range(2)]
                for half in range(2):
                    nc.gpsimd.dma_start(
                        out=xbf[half][:, :, :],
                        in_=xT.ap().rearrange("(kt p) s -> p kt s", p=128)[
                            :, KH * half:KH * (half + 1), 512 * sc:512 * (sc + 1)])

                p_l1 = psum_small.tile([80, 512], DT.float32, tag="p_small", name="p_l1")
                for half in range(2):
                    for j in range(KH):
                        kt = KH * half + j
                        nc.tensor.matmul(p_l1[:, :], lhsT=acomb_sb[:, kt, :], rhs=xbf[half][:, j, :],
                                         start=(kt == 0), stop=(kt == KT - 1), skip_group_check=True)
                l1q_sb = work.tile([16, 512], DT.bfloat16, tag="l1q")
                l1k_sb = work.tile([16, 512], DT.bfloat16, tag="l1k")
                l1v_sb = work.tile([16, 512], DT.bfloat16, tag="l1v")
                nc.scalar.activation(out=l1q_sb[:, :], in_=p_l1[0:16, :], func=AF.Copy, scale=LORA_SCALING)
                nc.scalar.activation(out=l1k_sb[:, :], in_=p_l1[32:48, :], func=AF.Copy, scale=LORA_SCALING)
                nc.scalar.activation(out=l1v_sb[:, :], in_=p_l1[64:80, :], func=AF.Copy, scale=LORA_SCALING)

                for ot in range(6):  # q0..q3, k, v -> psum [o=128, s=512]
                    ps = psum.tile([128, 512], DT.float32, tag="p_proj", name=f"ps{sc}_{ot}")
                    for half in range(2):
                        for j in range(KH):
                            kt = KH * half + j
                            nc.tensor.matmul(ps[:, :], lhsT=wt_qkv[ot][:, kt, :], rhs=xbf[half][:, j, :],
                                             start=(kt == 0), stop=False, skip_group_check=True)
                    if ot < 4:
                        nc.tensor.matmul(ps[:, :], lhsT=bq_sb[:, 128 * ot:128 * (ot + 1)],
                                         rhs=l1q_sb[:, :], start=False, stop=True, skip_group_check=True)
                    elif ot == 4:
                        nc.tensor.matmul(ps[:, :], lhsT=bk_sb[:, :], rhs=l1k_sb[:, :],
                                         start=False, stop=True, skip_group_check=True)
                    else:
                        nc.tensor.matmul(ps[:, :], lhsT=bv_sb[:, :], rhs=l1v_sb[:, :],
                                         start=False, stop=True, skip_group_check=True)
                    raw = work.tile([128, 512], DT.bfloat16, tag="praw", name=f"raw{sc}_{ot}")
                    nc.scalar.activation(out=raw[:, :], in_=ps[:, :], func=AF.Copy)
                    if ot < 5:
                        rope(qT[ot] if ot < 4 else kT, raw, sc)
                    else:
                        nc.sync.dma_start_transpose(out=v_sb[:, 4 * sc:4 * (sc + 1), :],
                                                    in_=raw[:, :])

            # ---- wo weight tiles (dequant interleaved with attention below) ----
            wt_wo = [wts.tile([128, KT, 128], DT.bfloat16, tag=f"wt{i}", name=f"wo{i}")
                     for i in range(N_WO)]

            # ---- attention (scores transposed; probsT feeds PV directly) ----
            # chunked over i: after each 512-column chunk finishes on all
            # heads, its AllGather is issued and the o-projection for that
            # chunk follows — comm and o-proj overlap later attention chunks.
            ag_in_c = [dram.tile([QO, 512], DT.bfloat16, tag=f"ag_in{ic}", name=f"ag_in{ic}")
                       for ic in range(n_sc)]
            ag_out_c = [dram.tile([NCORES * QO, 512], DT.bfloat16, tag=f"ag_out{ic}",
                                  name=f"ag_out{ic}", addr_space="Shared")
                        for ic in range(n_sc)]

            def attn_chunk(h, ic):
                i0 = 512 * ic
                a_ps = psum_acc.tile([128, 512], DT.float32, tag="p_attn", name=f"a{h}_{ic}")
                r_ps = psum_acc.tile([1, 512], DT.float32, tag="p_rows", name=f"r{h}_{ic}")
                jmax = 4 * ic + 3
                for jt in range(jmax + 1):
                    off = max(0, 128 * (jt - 4 * ic))
                    s_ps = psum.tile([128, 512], DT.float32, tag="p_proj", name=f"s{h}_{ic}_{jt}")
                    nc.tensor.matmul(s_ps[:, off:512], lhsT=kT[:, 128 * jt:128 * (jt + 1)],
                                     rhs=qT[h][:, i0 + off:i0 + 512], start=True, stop=True,
                                     skip_group_check=True)
                    if jt >= 4 * ic:  # diagonal block: additive causal mask (transposed)
                        nc.vector.tensor_tensor(out=s_ps[:, off:off + 128],
                                                in0=s_ps[:, off:off + 128],
                                                in1=maskT_sb[:, :], op=ALU.add)
                    pT = work.tile([128, 512], DT.bfloat16, tag="probsT", name=f"pT{h}_{ic}_{jt}")
                    nc.scalar.activation(out=pT[:, off:512], in_=s_ps[:, off:512],
                                         func=AF.Exp, scale=SM_SCALE)
                    nc.tensor.matmul(r_ps[:, off:512], lhsT=ones_col[:, :], rhs=pT[:, off:512],
                                     start=(jt == 0), stop=(jt == jmax), skip_group_check=True)
                    nc.tensor.matmul(a_ps[:, off:512], lhsT=v_sb[:, jt, :], rhs=pT[:, off:512],
                                     start=(jt == 0), stop=(jt == jmax), skip_group_check=True)
                rs_sb = work.tile([1, 512], DT.float32, tag="rs")
                nc.scalar.activation(out=rs_sb[:, :], in_=r_ps[:, :], func=AF.Copy)
                rc_sb = work.tile([1, 512], DT.float32, tag="rc")
                nc.vector.reciprocal(rc_sb[:, :], rs_sb[:, :])
                rcb = work.tile([1, 512], DT.bfloat16, tag="rcb")
                nc.vector.tensor_copy(rcb[:, :], rc_sb[:, :])
                b_ps = psum_small.tile([128, 512], DT.float32, tag="p_small", name=f"b{h}_{ic}")
                nc.tensor.matmul(b_ps[:, :], lhsT=ones_row[:, :], rhs=rcb[:, :],
                                 start=True, stop=True, skip_group_check=True)
                bc_sb = work.tile([128, 512], DT.float32, tag="bc")
                nc.vector.tensor_copy(bc_sb[:, :], b_ps[:, :])
                at_sb = work.tile([128, 512], DT.bfloat16, tag="attn")
                nc.vector.tensor_tensor(out=at_sb[:, :], in0=a_ps[:, :], in1=bc_sb[:, :],
                                        op=ALU.mult)
                nc.sync.dma_start(out=ag_in_c[ic][128 * h:128 * (h + 1), :], in_=at_sb[:, :])

            def oproj_chunk(sc):
                ag_r = ag_out_c[sc].rearrange("(kt p) s -> p kt s", p=128)
                gbf = [xin.tile([128, KH, 512], DT.bfloat16, tag="xbf", name=f"gbf{sc}_{h}")
                       for h in range(2)]
                for half in range(2):
                    nc.sync.dma_start(out=gbf[half][:, :, :],
                                      in_=ag_r[:, KH * half:KH * (half + 1), :])
                p_lo = psum_small.tile([16, 512], DT.float32, tag="p_small", name=f"p_lo{sc}")
                for half in range(2):
                    for j in range(KH):
                        kt = KH * half + j
                        nc.tensor.matmul(p_lo[:, :], lhsT=ao_sb[:, kt, :], rhs=gbf[half][:, j, :],
                                         start=(kt == 0), stop=(kt == KT - 1), skip_group_check=True)
                lo_sb = work.tile([16, 512], DT.bfloat16, tag="l1", name=f"lo{sc}")
                nc.scalar.activation(out=lo_sb[:, :], in_=p_lo[:, :], func=AF.Copy, scale=LORA_SCALING)
                for ot in range(N_WO):
                    ps = psum.tile([128, 512], DT.float32, tag="p_proj", name=f"py{sc}_{ot}")
                    for half in range(2):
                        for j in range(KH):
                            kt = KH * half + j
                            nc.tensor.matmul(ps[:, :], lhsT=wt_wo[ot][:, kt, :], rhs=gbf[half][:, j, :],
                                             start=(kt == 0), stop=False, skip_group_check=True)
                    nc.tensor.matmul(ps[:, :], lhsT=bo_sb[:, 128 * ot:128 * (ot + 1)], rhs=lo_sb[:, :],
                                     start=False, stop=True, skip_group_check=True)
                    y_sb = work.tile([128, 512], DT.float32, tag="y", name=f"y{sc}_{ot}")
                    nc.scalar.activation(out=y_sb[:, :], in_=ps[:, :], func=AF.Copy)
                    nc.sync.dma_start(out=out[128 * ot:128 * (ot + 1), 512 * sc:512 * (sc + 1)],
                                      in_=y_sb[:, :])

            # wo dequant is front-loaded into the first attention chunks; the
            # o-projection of early chunks starts before the last attention
            # chunk so the final AllGather hides behind compute.
            wo_sched = [[] for _ in range(n_sc)]
            for i in range(N_WO):
                wo_sched[min(i // 2, n_sc - 1)].append(i)
            op_sched = [[] for _ in range(n_sc)]
            for sc in range(n_sc):
                op_sched[min(sc + 2, n_sc - 1)].append(sc)
            for ic in range(n_sc):
                for h in range(HPC):
                    attn_chunk(h, ic)
                nc.gpsimd.collective_compute(
                    "AllGather", ALU.bypass,
                    replica_groups=[list(range(NCORES))],
                    ins=[ag_in_c[ic][:, :].opt()],
                    outs=[ag_out_c[ic][:, :].opt()],
                )
                for i in wo_sched[ic]:
                    dequant_otile(N_QKV + i, wt_wo[i])
                for sc in op_sched[ic]:
                    oproj_chunk(sc)
    nc.compile()
    return nc


# ---------------- host side ----------------

_PERM128 = np.concatenate([np.arange(0, 128, 2), np.arange(1, 128, 2)])


def _perm_rows(n_heads):
    return np.concatenate([128 * h + _PERM128 for h in range(n_heads)])


def shard_inputs(x, cos, sin, mask, wq_idx, wq_absmax, wq_A, wq_B,
                 wk_idx, wk_absmax, wk_A, wk_B, wv_idx, wv_absmax, wv_A, wv_B,
                 wo_idx, wo_absmax, wo_A, wo_B):
    s = np.asarray(x).shape[1]
    xT = np.ascontiguousarray(np.asarray(x, dtype=np.float32).reshape(s, D).T)
    cosT = np.asarray(cos, dtype=np.float32).T
    sinT = np.asarray(sin, dtype=np.float32).T
    # RoPE pairs de-interleaved to [evens(64); odds(64)]: both halves use the
    # same per-pair angle table.
    cosE = np.ascontiguousarray(np.concatenate([cosT, cosT], axis=0))
    sinE = np.ascontiguousarray(np.concatenate([sinT, sinT], axis=0))
    maskTd = np.ascontiguousarray(np.asarray(mask, dtype=np.float32)[0:128, 0:128].T)

    am_q = np.asarray(wq_absmax, dtype=np.float32).reshape(D, BLOCK)
    am_k = np.asarray(wk_absmax, dtype=np.float32).reshape(N_KV_HEADS * HEAD_DIM, BLOCK)
    am_v = np.asarray(wv_absmax, dtype=np.float32).reshape(N_KV_HEADS * HEAD_DIM, BLOCK)
    am_o = np.asarray(wo_absmax, dtype=np.float32).reshape(D, BLOCK)
    pq = _perm_rows(HPC)
    pk = _perm_rows(1)
    acomb = np.zeros((80, D), dtype=np.float32)
    acomb[0:16] = wq_A
    acomb[32:48] = wk_A
    acomb[64:80] = wv_A
    acombT = np.ascontiguousarray(acomb.T)
    aoT = np.ascontiguousarray(np.asarray(wo_A, dtype=np.float32).T)

    in_maps = []
    for m in range(NCORES):
        qs, ks = slice(QO * m, QO * (m + 1)), slice(KVO * m, KVO * (m + 1))
        idx = np.concatenate([
            np.asarray(wq_idx)[qs][pq], np.asarray(wk_idx)[ks][pk],
            np.asarray(wv_idx)[ks], np.asarray(wo_idx)[qs]], axis=0)
        am = np.concatenate([am_q[qs][pq], am_k[ks][pk], am_v[ks], am_o[qs]], axis=0)
        in_maps.append({
            "xT": xT,
            "qkvo_idx": np.ascontiguousarray(idx.astype(np.int32)),
            "qkvo_am": np.ascontiguousarray(am),
            "acombT": acombT,
            "aoT": aoT,
            "bqT": np.ascontiguousarray(np.asarray(wq_B, dtype=np.float32)[qs][pq].T),
            "bkT": np.ascontiguousarray(np.asarray(wk_B, dtype=np.float32)[ks][pk].T),
            "bvT": np.ascontiguousarray(np.asarray(wv_B, dtype=np.float32)[ks].T),
            "boT": np.ascontiguousarray(np.asarray(wo_B, dtype=np.float32)[qs].T),
            "cosT": cosE,
            "sinT": sinE,
            "maskTd": maskTd,
        })
    return in_maps


def assemble(results, s):
    y = np.concatenate([np.asarray(results[m]["out"]).T for m in range(NCORES)], axis=1)
    return np.ascontiguousarray(y.reshape(1, s, D).astype(np.float32))


_CACHED = {}


def _install_ntff_hook():
    """The agent image's antenv lacks axon_hooks; synthesize it so
    run_bass_kernel_spmd(trace=True) can capture NTFF profiles."""
    import sys as _sys
    import types as _types
    if "antenv.axon_hooks" in _sys.modules:
        return
    try:
        from trn_agent_boot.trn_boot import _ntff_profile_via_ctypes
        hook = _ntff_profile_via_ctypes("/opt/axon/libaxon_pjrt.so")
    except Exception:
        hook = None
    mod = _types.ModuleType("antenv.axon_hooks")
    mod._hook = hook
    mod.get_axon_ntff_profile_hook = lambda: mod._hook
    mod.set_axon_ntff_profile_hook = lambda h: setattr(mod, "_hook", h)
    _sys.modules["antenv.axon_hooks"] = mod
    try:
        import antenv
        antenv.axon_hooks = mod
    except Exception:
        pass


def kernel(**inputs):
    from concourse.bass_utils import run_bass_kernel_spmd

    trace = bool(_CACHED.pop("trace", False))
    if trace:
        _install_ntff_hook()
    in_maps = shard_inputs(**inputs)
    s = np.asarray(inputs["x"]).shape[1]
    key = ("nc", s)
    if key not in _CACHED:
        _CACHED[key] = build_graph(s)
    nc = _CACHED[key]
    res = run_bass_kernel_spmd(nc, in_maps, core_ids=list(range(NCORES)), trace=trace)
    if trace:
        _CACHED["last_res"] = res
    return assemble(res.results, s)
